# revision 22
# baseline (speedup 1.0000x reference)
"""Trainium2 Bass kernel for nn_MultiHeadMLPAttentionModel.

Model: per (b, n) point: pairwise = [radar_b(4), pt(2)] (radar constant over n).
  h1 = relu(pairwise @ enc_w1 + enc_b1)            [B,N,64]
  pf = h1 @ enc_w2 + enc_b2                        [B,N,64]
  sh = relu(einsum('bnf,hfd', pairwise, sc_w1) + sc_b1)
  logits = einsum('bnhd,hd', sh, sc_w2) + sc_b2    [B,N,4]
  w = softmax(logits, axis=n)
  ctx = einsum('bnh,bnd', w, pf)  -> out MLP -> [B]

Key algebraic restructurings used here:
  * pooling commutes with the (linear) second encoder layer since softmax
    weights sum to 1:  ctx = (sum_n w * h1) @ enc_w2 + enc_b2.  This removes
    the N-scale enc2 matmul entirely.
  * sc_b2 is constant over n, so it drops out of the softmax.
  * the radar part of pairwise is constant over n, so all layer-1 radar
    contributions fold into per-b bias vectors (computed on host: ~200 KFLOP
    of the model's 13 GFLOP).
  * softmax is computed without max-subtraction (logits are O(1) for this
    model; exp is evaluated in fp32) and normalization is deferred: the
    pooling matmul accumulates unnormalized sum_n exp(l)*h1 plus sum_n exp(l)
    (via an appended ones column), and the division happens once per b.

Sharding: pure data parallel over B: 8 cores x 16 rows each.  One SPMD Bass
program; per-core inputs differ only in data.
"""

import numpy as np

import concourse.bass as bass
import concourse.tile as tile
from concourse import bacc, mybir

B, N, HID, HEADS = 128, 8192, 64, 4
NCORES = 8
BPC = B // NCORES  # 16 batch rows per core
CHUNK = 512
NCH = N // CHUNK  # 16
NB = N // 128  # 64 point-blocks of 128

F32 = mybir.dt.float32
BF16 = mybir.dt.bfloat16
AF = mybir.ActivationFunctionType
ALU = mybir.AluOpType


def build_nc(reps=1, phases="ATPD"):
    from contextlib import ExitStack

    nc = bacc.Bacc()
    f32 = F32

    xp_d = nc.dram_tensor("xp", [BPC, 6, N], BF16, kind="ExternalInput")
    xpa_d = nc.dram_tensor("xpa", [NCH, 4, BPC * CHUNK], BF16, kind="ExternalInput")
    # replicated at 4 partition-group offsets for PE row-tiling
    cb1_d = nc.dram_tensor("cb1", [128, BPC], f32, kind="ExternalInput")
    cb2_d = nc.dram_tensor("cb2", [128, BPC], f32, kind="ExternalInput")
    wp_d = nc.dram_tensor("wp", [4, 256], BF16, kind="ExternalInput")
    w2a_d = nc.dram_tensor("w2a", [128, BPC * 128], BF16, kind="ExternalInput")
    w2b_d = nc.dram_tensor("w2b", [128, BPC * 128], BF16, kind="ExternalInput")
    wenm_d = nc.dram_tensor("wenm", [6, BPC * 65], BF16, kind="ExternalInput")
    ew2b_d = nc.dram_tensor("ew2b", [65, 64], f32, kind="ExternalInput")
    ow1_d = nc.dram_tensor("ow1", [64, 256], f32, kind="ExternalInput")
    ob1_d = nc.dram_tensor("ob1", [1, 64], f32, kind="ExternalInput")
    w2o_d = nc.dram_tensor("w2o", [65, 1], f32, kind="ExternalInput")
    id128_d = nc.dram_tensor("id128", [128, 128], BF16, kind="ExternalInput")
    on16_d = nc.dram_tensor("on16", [1, BPC], f32, kind="ExternalInput")
    out_d = nc.dram_tensor("out", [BPC], f32, kind="ExternalOutput")

    with tile.TileContext(nc) as tc, ExitStack() as ctx:
        consts = ctx.enter_context(tc.tile_pool(name="consts", bufs=1))

        def cload(dram, shape, nm, dt=f32):
            t = consts.tile(shape, dt, name=nm, tag=nm)
            nc.sync.dma_start(t[:], dram[:])
            return t

        # wp replicated at partition offsets {0,32,64,96} so the four K=4
        # score matmuls of a b-pair can run concurrently in distinct PE
        # row-groups (tile_position row tiling).
        wp_s = consts.tile([100, 256], BF16, name="wp_s", tag="wp_s")
        for u in range(4):
            nc.sync.dma_start(wp_s[32 * u : 32 * u + 4, :], wp_d[:])
        cb1_s = cload(cb1_d, [128, BPC], "cb1_s")
        cb2_s = cload(cb2_d, [128, BPC], "cb2_s")
        w2a_s = cload(w2a_d, [128, BPC * 128], "w2a_s", BF16)
        w2b_s = cload(w2b_d, [128, BPC * 128], "w2b_s", BF16)
        ew2b_s = cload(ew2b_d, [65, 64], "ew2b_s")
        ow1_s = cload(ow1_d, [64, 256], "ow1_s")
        ob1_s = cload(ob1_d, [1, 64], "ob1_s")
        w2o_s = cload(w2o_d, [65, 1], "w2o_s")
        id128_s = cload(id128_d, [128, 128], "id128_s", BF16)
        on16_s = cload(on16_d, [1, BPC], "on16_s")
        # wenm replicated at partition offsets {0,32} for 2-way row-tiled
        # encoder matmuls
        wenm_s = consts.tile([38, BPC * 65], BF16, name="wenm_s", tag="wenm_s")
        for r in range(2):
            nc.sync.dma_start(wenm_s[32 * r : 32 * r + 6, :], wenm_d[:])

        # n-major exp(logits): block t occupies cols [t*128, (t+1)*128);
        # within a block: partition p = n offset, col = 4*b + h
        enm = consts.tile([128, NB * 128], BF16, name="enm", tag="enm")
        ctxnT = consts.tile([65, 64], f32, name="ctxnT", tag="ctxnT")
        obuf = consts.tile([65, BPC], f32, name="obuf", tag="obuf")
        fct = consts.tile([64, 64], f32, name="fct", tag="fct")
        res = consts.tile([1, BPC], f32, name="res", tag="res")
        ones64 = consts.tile([1, 64], f32, name="ones64", tag="ones64")
        rz64 = consts.tile([1, 64], f32, name="rz64", tag="rz64")
        rbc_sb = consts.tile([64, 64], f32, name="rbc_sb", tag="rbc_sb")
        nc.vector.memset(obuf[64:65, :], 1.0)
        nc.vector.memset(ones64[:], 1.0)

        if "A" not in phases:
            nc.vector.memset(enm[:, 0:8], 0.0)
        for _rep in range(reps):
            _build_body(
                nc, tc, xp_d, xpa_d, out_d,
                wp_s, cb1_s, cb2_s, w2a_s, w2b_s, wenm_s, ew2b_s, ow1_s,
                ob1_s, w2o_s, id128_s, on16_s,
                enm, ctxnT, obuf, fct, res, ones64, rz64, rbc_sb, phases,
            )

    if not nc.is_finalized():
        nc.finalize()
    return nc


def _build_body(
    nc, tc, xp_d, xpa_d, out_d,
    wp_s, cb1_s, cb2_s, w2a_s, w2b_s, wenm_s, ew2b_s, ow1_s,
    ob1_s, w2o_s, id128_s, on16_s,
    enm, ctxnT, obuf, fct, res, ones64, rz64, rbc_sb, phases="ATPD",
):
    from contextlib import ExitStack

    f32 = F32
    if "A" in phases:
        # ---- Phase A: score-net hidden + logits (feature-major) ----------
        with ExitStack() as pctx:
            xpool = pctx.enter_context(tc.tile_pool(name="xpA", bufs=3))
            shpool = pctx.enter_context(tc.tile_pool(name="shp", bufs=8))
            epool = pctx.enter_context(tc.tile_pool(name="ep", bufs=2))
            psA = pctx.enter_context(tc.tile_pool(name="psA", bufs=4, space="PSUM"))
            psL = pctx.enter_context(tc.tile_pool(name="psL", bufs=2, space="PSUM"))
            psT = pctx.enter_context(tc.tile_pool(name="psT", bufs=2, space="PSUM"))

            xpcs = {}

            def load_xpc(c):
                # point data replicated at 4 partition-group offsets for
                # row-tiled matmuls
                t = xpool.tile([100, BPC * CHUNK], BF16, name="xpc", tag="xpc")
                for u in range(4):
                    nc.sync.dma_start(t[32 * u : 32 * u + 4, :], xpa_d[c])
                xpcs[c] = t

            DEPTH = 2  # software-pipeline depth (in b-pairs)
            lg_done = {}

            def expose(c):
                # exp of chunk c's logits, then transpose its 4 blocks n-major
                lg = lg_done.pop(c)
                e_c = epool.tile([128, CHUNK], BF16, name="e_c", tag="e_c")
                nc.scalar.activation(e_c[:], lg[:], AF.Exp)
                for j in range(CHUNK // 128):
                    t = c * (CHUNK // 128) + j
                    t_ps = psT.tile([128, 128], BF16, name="t_ps", tag="tp")
                    nc.tensor.transpose(
                        t_ps[:], e_c[:, j * 128 : (j + 1) * 128], id128_s[:]
                    )
                    nc.vector.tensor_copy(
                        out=enm[:, t * 128 : (t + 1) * 128], in_=t_ps[:]
                    )

            load_xpc(0)
            if NCH > 1:
                load_xpc(1)
            for c in range(NCH):
                if c + 2 < NCH:
                    load_xpc(c + 2)
                if c > 0:
                    expose(c - 1)
                xpc = xpcs.pop(c)
                lg_ps = psL.tile([128, CHUNK], f32, name="lg_ps", tag="lg")
                pend = []

                def drain_lg(lg_ps=lg_ps):
                    # logits for a b-pair: four K=128, M=128 matmuls
                    # accumulating into disjoint columns (4b+h) of lg_ps.
                    # M=128 (vs the minimal 32) costs nothing in streamed
                    # columns but keeps the whole PE array toggling, which
                    # holds the HAM clock-gate at 2.4 GHz.
                    j, sbs = pend.pop(0)
                    for u in range(4):
                        w2 = w2a_s if u % 2 == 0 else w2b_s
                        b = 2 * j + u // 2
                        nc.tensor.matmul(
                            lg_ps[:],
                            w2[:, b * 128 : (b + 1) * 128],
                            sbs[u][:],
                            start=(j == 0 and u == 0),
                            stop=(j == BPC // 2 - 1 and u == 3),
                            skip_group_check=True,
                        )

                for j in range(BPC // 2):
                    b0 = 2 * j
                    # four K=4 score matmuls (two b's x two head-pairs) run
                    # concurrently in four PE row-groups
                    sh_ps, sh_sb = [], []
                    for u in range(4):
                        b = b0 + u // 2
                        xb = xpc[
                            32 * u : 32 * u + 4, b * CHUNK : (b + 1) * CHUNK
                        ]
                        wslice = wp_s[
                            32 * u : 32 * u + 4,
                            (u % 2) * 128 : (u % 2) * 128 + 128,
                        ]
                        ps = psA.tile([128, CHUNK], f32, name="sh_ps", tag="sh")
                        nc.tensor.matmul(
                            ps[:], wslice, xb, start=True, stop=True,
                            tile_position=(32 * u, 0),
                        )
                        sh_ps.append(ps)
                    for u in range(4):
                        b = b0 + u // 2
                        cb = cb1_s if u % 2 == 0 else cb2_s
                        sb = shpool.tile([128, CHUNK], BF16, name="sh_sb", tag="shs")
                        if u % 2 == j % 2:
                            nc.scalar.activation(
                                sb[:], sh_ps[u][:], AF.Relu, bias=cb[:, b : b + 1]
                            )
                        else:
                            nc.vector.tensor_scalar(
                                sb[:], sh_ps[u][:], cb[:, b : b + 1], 0.0,
                                ALU.add, ALU.max,
                            )
                        sh_sb.append(sb)
                    # drain order within a pair: (s1 b0, s2 b0, s1 b1, s2 b1)
                    pend.append((j, sh_sb))
                    while len(pend) > DEPTH:
                        drain_lg()
                while pend:
                    drain_lg()
                lg_done[c] = lg_ps
            expose(NCH - 1)

    if "P" in phases:
        # ---- Phase P: n-major encoder hidden + weighted pooling ----------
        with ExitStack() as pctx:
            xbpool = pctx.enter_context(tc.tile_pool(name="xpC", bufs=2))
            h1pool = pctx.enter_context(tc.tile_pool(name="h1p", bufs=3))
            psH = pctx.enter_context(tc.tile_pool(name="psH", bufs=2, space="PSUM"))
            psC = pctx.enter_context(tc.tile_pool(name="psC", bufs=2, space="PSUM"))
            GB = 8  # blocks per h1/pool group (2 row-tiles x 4)
            xpbs = {}

            def load_xpb(b):
                # point rows replicated at partition offsets {0,32} for 2-way
                # row-tiled encoder matmuls
                t = xbpool.tile([38, N], BF16, name="xpb", tag="xpb")
                for r in range(2):
                    nc.sync.dma_start(t[32 * r : 32 * r + 6, :], xp_d[b])
                xpbs[b] = t

            load_xpb(0)
            for b in range(BPC):
                if b + 1 < BPC:
                    load_xpb(b + 1)
                xpb = xpbs.pop(b)
                c1_ps = psC.tile([65, 4], f32, name="c1_ps", tag="c1")
                hpend = []

                def drain_pool(c1_ps=c1_ps, b=b):
                    # pooling: stationary = h1 block, moving = 4 exp columns
                    g, h1_sb = hpend.pop(0)
                    for k in range(GB):
                        t = g * GB + k
                        nc.tensor.matmul(
                            c1_ps[:],
                            h1_sb[:, k * 65 : (k + 1) * 65],
                            enm[:, t * 128 + 4 * b : t * 128 + 4 * b + 4],
                            start=(t == 0),
                            stop=(t == NB - 1),
                            skip_group_check=True,
                        )

                for g in range(NB // GB):
                    h1_ps = [
                        psH.tile([128, 4 * 65], f32, name="h1_ps", tag="h1")
                        for _ in range(2)
                    ]
                    for r in range(2):
                        for jj in range(4):
                            t = g * GB + r * 4 + jj
                            nc.tensor.matmul(
                                h1_ps[r][:, jj * 65 : (jj + 1) * 65],
                                xpb[32 * r : 32 * r + 6, t * 128 : (t + 1) * 128],
                                wenm_s[32 * r : 32 * r + 6, b * 65 : (b + 1) * 65],
                                start=True,
                                stop=True,
                                tile_position=(32 * r, 0),
                                skip_group_check=True,
                            )
                    h1_sb = h1pool.tile([128, GB * 65], BF16, name="h1_sb", tag="h1s")
                    for r in range(2):
                        dst = h1_sb[:, r * 260 : (r + 1) * 260]
                        if r == g % 2:
                            nc.vector.tensor_scalar(
                                dst, h1_ps[r][:], 0.0, None, ALU.max
                            )
                        else:
                            nc.scalar.activation(dst, h1_ps[r][:], AF.Relu)
                    hpend.append((g, h1_sb))
                    if len(hpend) > 1:
                        drain_pool()
                while hpend:
                    drain_pool()
                # c1_ps rows 0:64 = unnormalized context (hidden-major), row
                # 64 = sum of exp; normalization deferred to phase D
                nc.vector.tensor_copy(
                    out=ctxnT[:, b * 4 : (b + 1) * 4], in_=c1_ps[:]
                )

    if "D" in phases:
        # ---- Phase D: pooled-context encoder layer 2 + output MLP --------
        with ExitStack() as pctx:
            psD = pctx.enter_context(tc.tile_pool(name="psD", bufs=1, space="PSUM"))
            # fct_un[:, 4b+h] = sum_e * (enc_w2.T ctx_norm + enc_b2)
            fct_ps = psD.tile([64, 64], f32, name="fct_ps", tag="fctp")
            nc.tensor.matmul(fct_ps[:], ew2b_s[:], ctxnT[:], start=True, stop=True)
            # normalize columns by 1/sum_e via a rank-1 broadcast matmul
            nc.vector.reciprocal(rz64[:], ctxnT[64:65, :])
            rbc_ps = psD.tile([64, 64], f32, name="rbc_ps", tag="rbcp")
            nc.tensor.matmul(rbc_ps[:], ones64[:], rz64[:], start=True, stop=True)
            nc.vector.tensor_copy(out=rbc_sb[:], in_=rbc_ps[:])
            nc.vector.scalar_tensor_tensor(
                fct[:], fct_ps[:], 1.0, rbc_sb[:], ALU.mult, ALU.mult
            )
            fct_bh = fct.rearrange("d (b h) -> d b h", h=HEADS)
            o1_ps = psD.tile([64, BPC], f32, name="o1_ps", tag="o1p")
            for h in range(HEADS):
                nc.tensor.matmul(
                    o1_ps[:],
                    ow1_s[:, h * 64 : (h + 1) * 64],
                    fct_bh[:, :, h],
                    start=(h == 0),
                    stop=False,
                    skip_group_check=True,
                )
            nc.tensor.matmul(
                o1_ps[:], ob1_s[:], on16_s[:], start=False, stop=True,
                skip_group_check=True,
            )
            nc.scalar.activation(obuf[0:64, :], o1_ps[:], AF.Relu)
            fin_ps = psD.tile([1, BPC], f32, name="fin_ps", tag="finp")
            nc.tensor.matmul(fin_ps[:], w2o_s[:], obuf[:], start=True, stop=True)
            nc.vector.tensor_copy(out=res[:], in_=fin_ps[:])
            nc.sync.dma_start(out_d.rearrange("(a n) -> a n", a=1), res[:])


def make_in_maps(inputs):
    """Host-side marshalling: slice B across cores and pack weights into the
    layouts the device program expects.

    bf16 note: the big streamed matmuls run in bf16.  To avoid systematic
    model-weight rounding, layer-1 weights are split hi/lo across extra
    contraction rows (w = hi + lo with both bf16); per-point input rounding
    is stochastic and averages out in the softmax pooling."""
    import ml_dtypes

    bf = ml_dtypes.bfloat16
    f = np.float32

    def split(a):
        hi = a.astype(bf)
        lo = (a - hi.astype(f)).astype(bf)
        return hi, lo
    radar = np.concatenate(
        [np.asarray(inputs["radar_xy"], f), np.asarray(inputs["radar_dir"], f)], axis=1
    )  # [B, 4]
    pts = np.asarray(inputs["pts"], f)
    enc_w1 = np.asarray(inputs["enc_w1"], f)
    enc_b1 = np.asarray(inputs["enc_b1"], f)
    enc_w2 = np.asarray(inputs["enc_w2"], f)
    enc_b2 = np.asarray(inputs["enc_b2"], f)
    sc_w1 = np.asarray(inputs["sc_w1"], f)
    sc_b1 = np.asarray(inputs["sc_b1"], f)
    sc_w2 = np.asarray(inputs["sc_w2"], f)
    out_w1 = np.asarray(inputs["out_w1"], f)
    out_b1 = np.asarray(inputs["out_b1"], f)
    out_w2 = np.asarray(inputs["out_w2"], f)
    out_b2 = np.asarray(inputs["out_b2"], f)

    # per-b layer-1 bias vectors (radar is constant over n)
    cb_sc = np.einsum("br,hrd->bhd", radar, sc_w1[:, :4, :]) + sc_b1  # [B, 4, 64]
    cb_enc = radar @ enc_w1[:4] + enc_b1  # [B, 64]

    # xp rows: [xh, yh, xh, yh, 1, 1] (bf16); rows 0-3 feed the weight-split
    # layer-1 matmuls, rows 4-5 carry the (split) bias contraction.
    xp = np.empty((B, 6, N), bf)
    xh = pts[:, :, 0].astype(bf)
    yh = pts[:, :, 1].astype(bf)
    xp[:, 0] = xh
    xp[:, 1] = yh
    xp[:, 2] = xh
    xp[:, 3] = yh
    xp[:, 4] = 1.0
    xp[:, 5] = 1.0

    # wp rows: [wxh, wyh, wxl, wyl] against xp rows [xh, yh, xh, yh]
    wp = np.empty((4, 256), bf)
    for h in range(HEADS):
        wxh, wxl = split(sc_w1[h, 4, :])
        wyh, wyl = split(sc_w1[h, 5, :])
        wp[0, h * 64 : (h + 1) * 64] = wxh
        wp[1, h * 64 : (h + 1) * 64] = wyh
        wp[2, h * 64 : (h + 1) * 64] = wxl
        wp[3, h * 64 : (h + 1) * 64] = wyl
    # heads 0,1 feed sh1 (wp cols 0:128), heads 2,3 feed sh2 (cols 128:256)

    # per-b logits stationaries [128, 128]: full-width M so the PE array
    # stays active (HAM warm); col 4b+h carries head h's weights, all other
    # columns zero.  w2a is applied to s1 tiles (heads 0,1), w2b to s2.
    w2a = np.zeros((128, BPC * 128), bf)
    w2b = np.zeros((128, BPC * 128), bf)
    for bl in range(BPC):
        base = bl * 128 + 4 * bl
        w2a[0:64, base + 0] = sc_w2[0]
        w2a[64:128, base + 1] = sc_w2[1]
        w2b[0:64, base + 2] = sc_w2[2]
        w2b[64:128, base + 3] = sc_w2[3]

    ew2b = np.concatenate([enc_w2, enc_b2[None, :]], axis=0)  # [65, 64]
    ow1 = np.empty((64, 256), f)
    for h in range(HEADS):
        ow1[:, h * 64 : (h + 1) * 64] = out_w1[h * 64 : (h + 1) * 64, :]
    ob1 = np.ascontiguousarray(out_b1[None, :])
    w2o = np.concatenate([out_w2, out_b2[None, :]], axis=0)  # [65, 1]
    id128 = np.eye(128, dtype=bf)
    on16 = np.ones((1, BPC), f)

    in_maps = []
    for c in range(NCORES):
        sl = slice(c * BPC, (c + 1) * BPC)
        cb1 = np.ascontiguousarray(cb_sc[sl, 0:2].reshape(BPC, 128).T)
        cb2 = np.ascontiguousarray(cb_sc[sl, 2:4].reshape(BPC, 128).T)
        # wenm rows [wxh, wyh, wxl, wyl, bh, bl] vs xp rows [xh, yh, xh, yh, 1, 1]
        wenm = np.zeros((6, BPC * 65), bf)
        exh, exl = split(enc_w1[4])
        eyh, eyl = split(enc_w1[5])
        for bl in range(BPC):
            s = slice(bl * 65, bl * 65 + 64)
            wenm[0, s] = exh
            wenm[1, s] = eyh
            wenm[2, s] = exl
            wenm[3, s] = eyl
            bh, blo = split(cb_enc[c * BPC + bl])
            wenm[4, s] = bh
            wenm[5, s] = blo
            wenm[4, bl * 65 + 64] = 1.0
        xpc_core = np.ascontiguousarray(xp[sl])
        xpa = np.ascontiguousarray(
            xpc_core[:, 0:4]
            .reshape(BPC, 4, NCH, CHUNK)
            .transpose(2, 1, 0, 3)
            .reshape(NCH, 4, BPC * CHUNK)
        )
        in_maps.append(
            dict(
                xp=xpc_core,
                xpa=xpa,
                cb1=cb1,
                cb2=cb2,
                wp=wp,
                w2a=w2a,
                w2b=w2b,
                wenm=wenm,
                ew2b=ew2b,
                ow1=ow1,
                ob1=ob1,
                w2o=w2o,
                id128=id128,
                on16=on16,
            )
        )
    return in_maps


_CACHE = {}


def _get_runner():
    """Build the Bass program once and a cached jitted PJRT executable over
    the 8 cores (shard_map along axis 0 of every input)."""
    if "runner" in _CACHE:
        return _CACHE["runner"]

    import jax
    from jax.sharding import Mesh, NamedSharding, PartitionSpec

    from concourse.bass2jax import (
        _bass_exec_p,
        install_neuronx_cc_hook,
        partition_id_tensor,
        shard_map,
    )

    nc = build_nc()
    _CACHE["nc"] = nc
    install_neuronx_cc_hook()
    partition_name = nc.partition_id_tensor.name if nc.partition_id_tensor else None
    in_names, out_names, out_avals = [], [], []
    for alloc in nc.m.functions[0].allocations:
        if not isinstance(alloc, mybir.MemoryLocationSet):
            continue
        name = alloc.memorylocations[0].name
        if alloc.kind == "ExternalInput":
            if name != partition_name:
                in_names.append(name)
        elif alloc.kind == "ExternalOutput":
            out_names.append(name)
            out_avals.append(
                jax.core.ShapedArray(tuple(alloc.tensor_shape), mybir.dt.np(alloc.dtype))
            )
    all_in_names = tuple(in_names + out_names)
    if partition_name is not None:
        all_in_names = all_in_names + (partition_name,)

    def _body(*args):
        operands = list(args)
        if partition_name is not None:
            operands.append(partition_id_tensor())
        return tuple(
            _bass_exec_p.bind(
                *operands,
                out_avals=tuple(out_avals),
                in_names=all_in_names,
                out_names=tuple(out_names),
                lowering_input_output_aliases=(),
                sim_require_finite=True,
                sim_require_nnan=True,
                nc=nc,
            )
        )

    devices = jax.devices()[:NCORES]
    mesh = Mesh(np.asarray(devices), ("core",))
    nin = len(in_names) + len(out_names)
    fn = jax.jit(
        shard_map(
            _body,
            mesh=mesh,
            in_specs=(PartitionSpec("core"),) * nin,
            out_specs=(PartitionSpec("core"),) * len(out_names),
            check_rep=False,
        ),
        keep_unused=True,
    )
    sharding = NamedSharding(mesh, PartitionSpec("core"))
    runner = (fn, sharding, in_names, out_avals)
    _CACHE["runner"] = runner
    return runner


def kernel(**inputs):
    import jax

    in_maps = make_in_maps(inputs)
    fn, sharding, in_names, out_avals = _get_runner()
    concat_in = [
        np.concatenate([np.asarray(in_maps[c][name]) for c in range(NCORES)], axis=0)
        for name in in_names
    ]
    concat_zeros = [
        np.zeros((NCORES * a.shape[0], *a.shape[1:]), a.dtype) for a in out_avals
    ]
    args = [jax.device_put(a, sharding) for a in (*concat_in, *concat_zeros)]
    (out,) = fn(*args)
    return np.asarray(out).reshape(B).astype(np.float32)



# revision 23
# speedup vs baseline: 1.6917x; 1.6917x over previous
"""Trainium2 Bass kernel for nn_MultiHeadMLPAttentionModel.

Model: per (b, n) point: pairwise = [radar_b(4), pt(2)] (radar constant over n).
  h1 = relu(pairwise @ enc_w1 + enc_b1)            [B,N,64]
  pf = h1 @ enc_w2 + enc_b2                        [B,N,64]
  sh = relu(einsum('bnf,hfd', pairwise, sc_w1) + sc_b1)
  logits = einsum('bnhd,hd', sh, sc_w2) + sc_b2    [B,N,4]
  w = softmax(logits, axis=n)
  ctx = einsum('bnh,bnd', w, pf)  -> out MLP -> [B]

Key algebraic restructurings:
  * pooling commutes with the (linear) second encoder layer since softmax
    weights sum to 1:  ctx = (sum_n w * h1) @ enc_w2 + enc_b2.
  * sc_b2 is constant over n, so it drops out of the softmax.
  * the radar part of pairwise folds into per-b layer-1 bias vectors
    (computed on host).
  * softmax normalization is deferred: pooling accumulates unnormalized
    sum_n exp(l)*h1 and sum_n exp(l); division happens once per (b,h) after
    the (linear) enc2 matmul in phase D.

Slot-packing (the TRN2-specific trick): the PE HAM clock-gate only counts
matmuls with a large contraction dim as "busy" — K=4 matmuls run at the cold
1.2 GHz clock forever.  So every point-data matmul here uses K=128: batch b's
per-point features live on partition rows 8b..8b+6 of a shared [128, N]
tensor, and each per-b stationary is zero outside its slot rows.  Streamed
columns are unchanged; the whole kernel stays at 2.4 GHz.

Sharding: pure data parallel over B: 8 cores x 16 rows each.
"""

import numpy as np

import concourse.bass as bass
import concourse.tile as tile
from concourse import bacc, mybir

B, N, HID, HEADS = 128, 8192, 64, 4
NCORES = 8
BPC = B // NCORES  # 16 batch rows per core
CHUNK = 512
NCH = N // CHUNK  # 16
NB = N // 128  # 64 point-blocks of 128

F32 = mybir.dt.float32
BF16 = mybir.dt.bfloat16
AF = mybir.ActivationFunctionType
ALU = mybir.AluOpType


def build_nc(reps=1, phases="APD"):
    from contextlib import ExitStack

    nc = bacc.Bacc()
    f32 = F32

    # xq: slot-packed points, row 8b+r = [xh, yh, xh, yh, 1, 1, 0, 0][r] of
    # batch b; col = position n
    xq_d = nc.dram_tensor("xq", [128, N], BF16, kind="ExternalInput")
    cb1_d = nc.dram_tensor("cb1", [128, BPC], f32, kind="ExternalInput")
    cb2_d = nc.dram_tensor("cb2", [128, BPC], f32, kind="ExternalInput")
    wp_d = nc.dram_tensor("wp", [4, 256], BF16, kind="ExternalInput")
    w2a_d = nc.dram_tensor("w2a", [128, BPC * 64], BF16, kind="ExternalInput")
    w2b_d = nc.dram_tensor("w2b", [128, BPC * 64], BF16, kind="ExternalInput")
    wenm_d = nc.dram_tensor("wenm", [6, BPC * 64], BF16, kind="ExternalInput")
    ew2b_d = nc.dram_tensor("ew2b", [65, 64], f32, kind="ExternalInput")
    ow1_d = nc.dram_tensor("ow1", [64, 256], f32, kind="ExternalInput")
    ob1_d = nc.dram_tensor("ob1", [1, 64], f32, kind="ExternalInput")
    w2o_d = nc.dram_tensor("w2o", [65, 1], f32, kind="ExternalInput")
    id64_d = nc.dram_tensor("id64", [64, 64], BF16, kind="ExternalInput")
    on16_d = nc.dram_tensor("on16", [1, BPC], f32, kind="ExternalInput")
    out_d = nc.dram_tensor("out", [BPC], f32, kind="ExternalOutput")

    with tile.TileContext(nc) as tc, ExitStack() as ctx:
        consts = ctx.enter_context(tc.tile_pool(name="consts", bufs=1))

        def cload(dram, shape, nm, dt=f32):
            t = consts.tile(shape, dt, name=nm, tag=nm)
            nc.sync.dma_start(t[:], dram[:])
            return t

        cb1_s = cload(cb1_d, [128, BPC], "cb1_s")
        cb2_s = cload(cb2_d, [128, BPC], "cb2_s")
        w2a_s = cload(w2a_d, [128, BPC * 64], "w2a_s", BF16)
        w2b_s = cload(w2b_d, [128, BPC * 64], "w2b_s", BF16)
        ew2b_s = cload(ew2b_d, [65, 64], "ew2b_s")
        ow1_s = cload(ow1_d, [64, 256], "ow1_s")
        ob1_s = cload(ob1_d, [1, 64], "ob1_s")
        w2o_s = cload(w2o_d, [65, 1], "w2o_s")
        id64_s = cload(id64_d, [64, 64], "id64_s", BF16)
        on16_s = cload(on16_d, [1, BPC], "on16_s")

        # slot-expanded stationaries: zero except each b's slot rows
        wpx_s = consts.tile([128, BPC * 256], BF16, name="wpx_s", tag="wpx_s")
        nc.vector.memset(wpx_s[:], 0.0)
        for b in range(BPC):
            nc.sync.dma_start(
                wpx_s[8 * b : 8 * b + 4, b * 256 : (b + 1) * 256], wp_d[:]
            )
        wex_s = consts.tile([128, BPC * 64], BF16, name="wex_s", tag="wex_s")
        nc.vector.memset(wex_s[:], 0.0)
        for b in range(BPC):
            nc.sync.dma_start(
                wex_s[8 * b : 8 * b + 6, b * 64 : (b + 1) * 64],
                wenm_d[:, b * 64 : (b + 1) * 64],
            )
        ones_s = consts.tile([128, 1], BF16, name="ones_s", tag="ones_s")
        nc.vector.memset(ones_s[:], 1.0)

        # resident slot-packed point data (16 KB/partition)
        xq_s = consts.tile([128, N], BF16, name="xq_s", tag="xq_s")
        for c in range(NCH):
            nc.sync.dma_start(
                xq_s[:, c * CHUNK : (c + 1) * CHUNK],
                xq_d[:, c * CHUNK : (c + 1) * CHUNK],
            )

        # n-major exp(logits): block t cols [t*64, (t+1)*64), within a block
        # partition p = n offset, col = 4*b + h
        enm = consts.tile([128, NB * 64], BF16, name="enm", tag="enm")
        ctxnT = consts.tile([65, 64], f32, name="ctxnT", tag="ctxnT")
        obuf = consts.tile([65, BPC], f32, name="obuf", tag="obuf")
        fct = consts.tile([64, 64], f32, name="fct", tag="fct")
        res = consts.tile([1, BPC], f32, name="res", tag="res")
        ones64 = consts.tile([1, 64], f32, name="ones64", tag="ones64")
        rz64 = consts.tile([1, 64], f32, name="rz64", tag="rz64")
        rbc_sb = consts.tile([64, 64], f32, name="rbc_sb", tag="rbc_sb")
        nc.vector.memset(obuf[64:65, :], 1.0)
        nc.vector.memset(ones64[:], 1.0)

        if "A" not in phases:
            nc.vector.memset(enm[:, 0:8], 0.0)
        for _rep in range(reps):
            _build_body(
                nc, tc, out_d,
                xq_s, wpx_s, wex_s, ones_s, cb1_s, cb2_s, w2a_s, w2b_s,
                ew2b_s, ow1_s, ob1_s, w2o_s, id64_s, on16_s,
                enm, ctxnT, obuf, fct, res, ones64, rz64, rbc_sb, phases,
            )

    if not nc.is_finalized():
        nc.finalize()
    return nc


def _build_body(
    nc, tc, out_d,
    xq_s, wpx_s, wex_s, ones_s, cb1_s, cb2_s, w2a_s, w2b_s,
    ew2b_s, ow1_s, ob1_s, w2o_s, id64_s, on16_s,
    enm, ctxnT, obuf, fct, res, ones64, rz64, rbc_sb, phases="APD",
):
    from contextlib import ExitStack

    f32 = F32
    if "A" in phases:
        # ---- Phase A: score-net hidden + logits (feature-major) ----------
        with ExitStack() as pctx:
            shpool = pctx.enter_context(tc.tile_pool(name="shp", bufs=8))
            epool = pctx.enter_context(tc.tile_pool(name="ep", bufs=2))
            psA = pctx.enter_context(tc.tile_pool(name="psA", bufs=4, space="PSUM"))
            psL = pctx.enter_context(tc.tile_pool(name="psL", bufs=2, space="PSUM"))
            psT = pctx.enter_context(tc.tile_pool(name="psT", bufs=2, space="PSUM"))

            DEPTH = 2  # software-pipeline depth: lg-MMs run DEPTH b's behind
            lg_done = {}

            def expose(c):
                # exp of chunk c's logits, then transpose its 4 blocks n-major
                lg = lg_done.pop(c)
                e_c = epool.tile([64, CHUNK], BF16, name="e_c", tag="e_c")
                nc.scalar.activation(e_c[:], lg[:], AF.Exp)
                for j in range(CHUNK // 128):
                    t = c * (CHUNK // 128) + j
                    t_ps = psT.tile([128, 64], BF16, name="t_ps", tag="tp")
                    nc.tensor.transpose(
                        t_ps[:], e_c[:, j * 128 : (j + 1) * 128], id64_s[:]
                    )
                    nc.vector.tensor_copy(
                        out=enm[:, t * 64 : (t + 1) * 64], in_=t_ps[:]
                    )

            for c in range(NCH):
                if c > 0:
                    expose(c - 1)
                xc = xq_s[:, c * CHUNK : (c + 1) * CHUNK]
                lg_ps = psL.tile([64, CHUNK], f32, name="lg_ps", tag="lg")
                pend = []

                def drain_lg(lg_ps=lg_ps):
                    b, s1, s2 = pend.pop(0)
                    nc.tensor.matmul(
                        lg_ps[:],
                        w2a_s[:, b * 64 : (b + 1) * 64],
                        s1[:],
                        start=(b == 0),
                        stop=False,
                        skip_group_check=True,
                    )
                    nc.tensor.matmul(
                        lg_ps[:],
                        w2b_s[:, b * 64 : (b + 1) * 64],
                        s2[:],
                        start=False,
                        stop=(b == BPC - 1),
                        skip_group_check=True,
                    )

                for b in range(BPC):
                    # K=128 slot-packed score matmuls: stationary is zero
                    # outside rows 8b..8b+4, so out = batch b's score hidden
                    sbs = []
                    for u in range(2):
                        ps = psA.tile([128, CHUNK], f32, name="sh_ps", tag="sh")
                        nc.tensor.matmul(
                            ps[:],
                            wpx_s[:, b * 256 + u * 128 : b * 256 + (u + 1) * 128],
                            xc,
                            start=True,
                            stop=True,
                            skip_group_check=True,
                        )
                        cb = cb1_s if u == 0 else cb2_s
                        sb = shpool.tile([128, CHUNK], BF16, name="sh_sb", tag="shs")
                        if u == b % 2:
                            nc.scalar.activation(
                                sb[:], ps[:], AF.Relu, bias=cb[:, b : b + 1]
                            )
                        else:
                            nc.vector.tensor_scalar(
                                sb[:], ps[:], cb[:, b : b + 1], 0.0,
                                ALU.add, ALU.max,
                            )
                        sbs.append(sb)
                    pend.append((b, sbs[0], sbs[1]))
                    while len(pend) > DEPTH:
                        drain_lg()
                while pend:
                    drain_lg()
                lg_done[c] = lg_ps
            expose(NCH - 1)

    if "P" in phases:
        # ---- Phase P: n-major encoder hidden + weighted pooling ----------
        with ExitStack() as pctx:
            h1pool = pctx.enter_context(tc.tile_pool(name="h1p", bufs=3))
            psH = pctx.enter_context(tc.tile_pool(name="psH", bufs=4, space="PSUM"))
            psC = pctx.enter_context(tc.tile_pool(name="psC", bufs=1, space="PSUM"))
            psE = pctx.enter_context(tc.tile_pool(name="psE", bufs=1, space="PSUM"))
            # all-b accumulators: ctx (hidden-major) and sum-of-exp
            c1_ps = psC.tile([64, 64], f32, name="c1_ps", tag="c1")
            se_ps = psE.tile([1, 64], f32, name="se_ps", tag="se")
            hpend = []

            def drain_pool():
                t, h1_sb = hpend.pop(0)
                # sum of exp for all (b,h) at once
                nc.tensor.matmul(
                    se_ps[:],
                    ones_s[:],
                    enm[:, t * 64 : (t + 1) * 64],
                    start=(t == 0),
                    stop=(t == NB - 1),
                    skip_group_check=True,
                )
                for b in range(BPC):
                    # pooling: stationary = batch b's h1 block, moving = its
                    # 4 exp columns; accumulates into c1_ps[:, 4b:4b+4]
                    nc.tensor.matmul(
                        c1_ps[:, 4 * b : 4 * b + 4],
                        h1_sb[:, b * 64 : (b + 1) * 64],
                        enm[:, t * 64 + 4 * b : t * 64 + 4 * b + 4],
                        start=(t == 0),
                        stop=(t == NB - 1),
                        skip_group_check=True,
                    )

            for t in range(NB):
                xb = xq_s[:, t * 128 : (t + 1) * 128]
                h1_ps = [
                    psH.tile([128, 512], f32, name="h1_ps", tag="h1")
                    for _ in range(2)
                ]
                for g in range(2):
                    # K=128 slot-packed encoder matmul: 8 b's of h1 at once
                    nc.tensor.matmul(
                        h1_ps[g][:],
                        xb,
                        wex_s[:, g * 512 : (g + 1) * 512],
                        start=True,
                        stop=True,
                        skip_group_check=True,
                    )
                h1_sb = h1pool.tile([128, 1024], BF16, name="h1_sb", tag="h1s")
                for g in range(2):
                    dst = h1_sb[:, g * 512 : (g + 1) * 512]
                    if g == t % 2:
                        nc.vector.tensor_scalar(
                            dst, h1_ps[g][:], 0.0, None, ALU.max
                        )
                    else:
                        nc.scalar.activation(dst, h1_ps[g][:], AF.Relu)
                hpend.append((t, h1_sb))
                if len(hpend) > 1:
                    drain_pool()
            while hpend:
                drain_pool()
            nc.vector.tensor_copy(out=ctxnT[0:64, :], in_=c1_ps[:])
            nc.vector.tensor_copy(out=ctxnT[64:65, :], in_=se_ps[:])

    if "D" in phases:
        # ---- Phase D: pooled-context encoder layer 2 + output MLP --------
        with ExitStack() as pctx:
            psD = pctx.enter_context(tc.tile_pool(name="psD", bufs=1, space="PSUM"))
            # fct_un[:, 4b+h] = sum_e * (enc_w2.T ctx_norm + enc_b2)
            fct_ps = psD.tile([64, 64], f32, name="fct_ps", tag="fctp")
            nc.tensor.matmul(fct_ps[:], ew2b_s[:], ctxnT[:], start=True, stop=True)
            # normalize columns by 1/sum_e via a rank-1 broadcast matmul
            nc.vector.reciprocal(rz64[:], ctxnT[64:65, :])
            rbc_ps = psD.tile([64, 64], f32, name="rbc_ps", tag="rbcp")
            nc.tensor.matmul(rbc_ps[:], ones64[:], rz64[:], start=True, stop=True)
            nc.vector.tensor_copy(out=rbc_sb[:], in_=rbc_ps[:])
            nc.vector.scalar_tensor_tensor(
                fct[:], fct_ps[:], 1.0, rbc_sb[:], ALU.mult, ALU.mult
            )
            fct_bh = fct.rearrange("d (b h) -> d b h", h=HEADS)
            o1_ps = psD.tile([64, BPC], f32, name="o1_ps", tag="o1p")
            for h in range(HEADS):
                nc.tensor.matmul(
                    o1_ps[:],
                    ow1_s[:, h * 64 : (h + 1) * 64],
                    fct_bh[:, :, h],
                    start=(h == 0),
                    stop=False,
                    skip_group_check=True,
                )
            nc.tensor.matmul(
                o1_ps[:], ob1_s[:], on16_s[:], start=False, stop=True,
                skip_group_check=True,
            )
            nc.scalar.activation(obuf[0:64, :], o1_ps[:], AF.Relu)
            fin_ps = psD.tile([1, BPC], f32, name="fin_ps", tag="finp")
            nc.tensor.matmul(fin_ps[:], w2o_s[:], obuf[:], start=True, stop=True)
            nc.vector.tensor_copy(out=res[:], in_=fin_ps[:])
            nc.sync.dma_start(out_d.rearrange("(a n) -> a n", a=1), res[:])


def make_in_maps(inputs):
    """Host-side marshalling: slice B across cores and pack weights into the
    layouts the device program expects.

    bf16 note: the big streamed matmuls run in bf16.  To avoid systematic
    model-weight rounding, layer-1 weights are split hi/lo across extra
    contraction rows (w = hi + lo with both bf16); per-point input rounding
    is stochastic and averages out in the softmax pooling."""
    import ml_dtypes

    bf = ml_dtypes.bfloat16
    f = np.float32

    def split(a):
        hi = a.astype(bf)
        lo = (a - hi.astype(f)).astype(bf)
        return hi, lo
    radar = np.concatenate(
        [np.asarray(inputs["radar_xy"], f), np.asarray(inputs["radar_dir"], f)], axis=1
    )  # [B, 4]
    pts = np.asarray(inputs["pts"], f)
    enc_w1 = np.asarray(inputs["enc_w1"], f)
    enc_b1 = np.asarray(inputs["enc_b1"], f)
    enc_w2 = np.asarray(inputs["enc_w2"], f)
    enc_b2 = np.asarray(inputs["enc_b2"], f)
    sc_w1 = np.asarray(inputs["sc_w1"], f)
    sc_b1 = np.asarray(inputs["sc_b1"], f)
    sc_w2 = np.asarray(inputs["sc_w2"], f)
    out_w1 = np.asarray(inputs["out_w1"], f)
    out_b1 = np.asarray(inputs["out_b1"], f)
    out_w2 = np.asarray(inputs["out_w2"], f)
    out_b2 = np.asarray(inputs["out_b2"], f)

    # per-b layer-1 bias vectors (radar is constant over n)
    cb_sc = np.einsum("br,hrd->bhd", radar, sc_w1[:, :4, :]) + sc_b1  # [B, 4, 64]
    cb_enc = radar @ enc_w1[:4] + enc_b1  # [B, 64]

    # wp rows: [wxh, wyh, wxl, wyl] against xq rows [xh, yh, xh, yh]
    wp = np.empty((4, 256), bf)
    for h in range(HEADS):
        wxh, wxl = split(sc_w1[h, 4, :])
        wyh, wyl = split(sc_w1[h, 5, :])
        wp[0, h * 64 : (h + 1) * 64] = wxh
        wp[1, h * 64 : (h + 1) * 64] = wyh
        wp[2, h * 64 : (h + 1) * 64] = wxl
        wp[3, h * 64 : (h + 1) * 64] = wyl
    # heads 0,1 feed s1 (wp cols 0:128), heads 2,3 feed s2 (cols 128:256)

    w2a = np.zeros((128, BPC * 64), bf)
    w2b = np.zeros((128, BPC * 64), bf)
    for bl in range(BPC):
        w2a[0:64, bl * 64 + 4 * bl + 0] = sc_w2[0]
        w2a[64:128, bl * 64 + 4 * bl + 1] = sc_w2[1]
        w2b[0:64, bl * 64 + 4 * bl + 2] = sc_w2[2]
        w2b[64:128, bl * 64 + 4 * bl + 3] = sc_w2[3]

    ew2b = np.concatenate([enc_w2, enc_b2[None, :]], axis=0)  # [65, 64]
    ow1 = np.empty((64, 256), f)
    for h in range(HEADS):
        ow1[:, h * 64 : (h + 1) * 64] = out_w1[h * 64 : (h + 1) * 64, :]
    ob1 = np.ascontiguousarray(out_b1[None, :])
    w2o = np.concatenate([out_w2, out_b2[None, :]], axis=0)  # [65, 1]
    id64 = np.eye(64, dtype=bf)
    on16 = np.ones((1, BPC), f)

    exh, exl = split(enc_w1[4])
    eyh, eyl = split(enc_w1[5])

    in_maps = []
    for c in range(NCORES):
        sl = slice(c * BPC, (c + 1) * BPC)
        cb1 = np.ascontiguousarray(cb_sc[sl, 0:2].reshape(BPC, 128).T)
        cb2 = np.ascontiguousarray(cb_sc[sl, 2:4].reshape(BPC, 128).T)
        # wenm rows [wxh, wyh, wxl, wyl, bh, bl] vs xq rows [xh,yh,xh,yh,1,1]
        wenm = np.zeros((6, BPC * 64), bf)
        for bl in range(BPC):
            s = slice(bl * 64, (bl + 1) * 64)
            wenm[0, s] = exh
            wenm[1, s] = eyh
            wenm[2, s] = exl
            wenm[3, s] = eyl
            bh, blo = split(cb_enc[c * BPC + bl])
            wenm[4, s] = bh
            wenm[5, s] = blo
        # slot-packed points [128, N]
        xq = np.zeros((128, N), bf)
        xh = pts[sl, :, 0].astype(bf)  # [BPC, N]
        yh = pts[sl, :, 1].astype(bf)
        for bl in range(BPC):
            xq[8 * bl + 0] = xh[bl]
            xq[8 * bl + 1] = yh[bl]
            xq[8 * bl + 2] = xh[bl]
            xq[8 * bl + 3] = yh[bl]
            xq[8 * bl + 4] = 1.0
            xq[8 * bl + 5] = 1.0
        in_maps.append(
            dict(
                xq=xq,
                cb1=cb1,
                cb2=cb2,
                wp=wp,
                w2a=w2a,
                w2b=w2b,
                wenm=wenm,
                ew2b=ew2b,
                ow1=ow1,
                ob1=ob1,
                w2o=w2o,
                id64=id64,
                on16=on16,
            )
        )
    return in_maps


_CACHE = {}


def _get_runner():
    """Build the Bass program once and a cached jitted PJRT executable over
    the 8 cores (shard_map along axis 0 of every input)."""
    if "runner" in _CACHE:
        return _CACHE["runner"]

    import jax
    from jax.sharding import Mesh, NamedSharding, PartitionSpec

    from concourse.bass2jax import (
        _bass_exec_p,
        install_neuronx_cc_hook,
        partition_id_tensor,
        shard_map,
    )

    nc = build_nc()
    _CACHE["nc"] = nc
    install_neuronx_cc_hook()
    partition_name = nc.partition_id_tensor.name if nc.partition_id_tensor else None
    in_names, out_names, out_avals = [], [], []
    for alloc in nc.m.functions[0].allocations:
        if not isinstance(alloc, mybir.MemoryLocationSet):
            continue
        name = alloc.memorylocations[0].name
        if alloc.kind == "ExternalInput":
            if name != partition_name:
                in_names.append(name)
        elif alloc.kind == "ExternalOutput":
            out_names.append(name)
            out_avals.append(
                jax.core.ShapedArray(tuple(alloc.tensor_shape), mybir.dt.np(alloc.dtype))
            )
    all_in_names = tuple(in_names + out_names)
    if partition_name is not None:
        all_in_names = all_in_names + (partition_name,)

    def _body(*args):
        operands = list(args)
        if partition_name is not None:
            operands.append(partition_id_tensor())
        return tuple(
            _bass_exec_p.bind(
                *operands,
                out_avals=tuple(out_avals),
                in_names=all_in_names,
                out_names=tuple(out_names),
                lowering_input_output_aliases=(),
                sim_require_finite=True,
                sim_require_nnan=True,
                nc=nc,
            )
        )

    devices = jax.devices()[:NCORES]
    mesh = Mesh(np.asarray(devices), ("core",))
    nin = len(in_names) + len(out_names)
    fn = jax.jit(
        shard_map(
            _body,
            mesh=mesh,
            in_specs=(PartitionSpec("core"),) * nin,
            out_specs=(PartitionSpec("core"),) * len(out_names),
            check_rep=False,
        ),
        keep_unused=True,
    )
    sharding = NamedSharding(mesh, PartitionSpec("core"))
    runner = (fn, sharding, in_names, out_avals)
    _CACHE["runner"] = runner
    return runner


def kernel(**inputs):
    import jax

    in_maps = make_in_maps(inputs)
    fn, sharding, in_names, out_avals = _get_runner()
    concat_in = [
        np.concatenate([np.asarray(in_maps[c][name]) for c in range(NCORES)], axis=0)
        for name in in_names
    ]
    concat_zeros = [
        np.zeros((NCORES * a.shape[0], *a.shape[1:]), a.dtype) for a in out_avals
    ]
    args = [jax.device_put(a, sharding) for a in (*concat_in, *concat_zeros)]
    (out,) = fn(*args)
    return np.asarray(out).reshape(B).astype(np.float32)
